# revision 8
# baseline (speedup 1.0000x reference)
"""Embedding-lookup kernel for TRN2 (8 NeuronCores, SPMD data-parallel).

Reference semantics (B=32, S=8192, D=512):
    table = concat(11 per-type tables, unknown_embed)   # [1726, 512] f32
    out[b, s] = table[flat_map[input_ids[b, s]]]

Strategy per core (batch-sharded, 4 rows = 32768 tokens/core):
  Host prep: compose flat_map into the fused table (np.take, mode='clip'
  = jnp.take default) and cast it to bf16 -> tbl [1725, 512] bf16 in HBM.
  bf16 halves the gather read traffic; max quantization rel-err is 2^-9
  ~= 0.2% of each element, far under the 2e-2 gate.

  Device: 32 chunks x 1024 tokens, NBUF-deep pipeline:
    1. SWDGE dma_gather: 1024 bf16 rows (1 KiB each) HBM -> SBUF, token
       order permuted so partition b holds 8 consecutive tokens.
    2. Upconvert bf16 -> f32 (SBUF -> SBUF): DVE for even buffers, ACT
       for odd buffers (chunk m -> buffer m%NBUF; NBUF even => fixed
       engine per buffer).
    3. HWDGE write: 128 descriptors x 16 KiB contiguous f32 -> out.
  HBM traffic/core: 32 MiB gather read + 64 MiB out write. The SWDGE
  descriptor ring is doubled (dynamic_dma_scratch_size=32768 -> 2048
  descs) so chunk m+1's decode overlaps chunk m's flight. ids prefix is
  one rank-4 DMA + one rank-4 DVE cast + 7 replicate copies (~8 us).
  Per-buffer semaphores throughout (DMA completions are unordered
  across instructions sharing a semaphore).
"""

import numpy as np
import ml_dtypes

import concourse.bass as bass
import concourse.bacc as bacc
import concourse.mybir as mybir
from concourse.bass_utils import run_bass_kernel_spmd
from concourse.library_config import mlp

# ---- problem dims (hardcoded per contract) ----
B, S, D = 32, 8192, 512
NCORES = 8
BPC = B // NCORES            # batch rows per core
T = BPC * S                  # tokens per core = 32768
VOCAB = 1725
CHUNK = 1024                 # tokens per main gather (SWDGE ring-capacity cap)
NCH = T // CHUNK             # 32 chunks
A = CHUNK // 128             # tokens per partition per chunk = 8
CC = CHUNK // 16 // A        # inner id groups per chunk = 8
NBUF = 6                     # main-loop buffers (even: fixed engine per buffer)

TAB_SPECS = [
    ("special_tab", 3), ("event_tab", 9), ("time_tab", 512), ("note_tab", 128),
    ("vel_tab", 32), ("prog_tab", 129), ("local_tab", 16), ("ccnum_tab", 128),
    ("ccval_tab", 128), ("progval_tab", 128), ("dur_tab", 512),
]

f32 = mybir.dt.float32
bf16 = mybir.dt.bfloat16
i32 = mybir.dt.int32
i16 = mybir.dt.int16


def build_nc(_nbuf: int = None, _scratch: int = 32768, _nq: int = 1) -> bacc.Bacc:
    global NBUF
    if _nbuf is not None:
        NBUF = _nbuf
    nc = bacc.Bacc("TRN2", target_bir_lowering=False, debug=False,
                   dynamic_dma_scratch_size=_scratch, num_swdge_queues=_nq)

    ids = nc.dram_tensor("ids", [T], i32, kind="ExternalInput")
    tbl = nc.dram_tensor("tbl", [VOCAB, D], bf16, kind="ExternalInput")
    out = nc.dram_tensor("out", [T, D], f32, kind="ExternalOutput")

    from contextlib import ExitStack
    with ExitStack() as stack:
        ec = stack.enter_context
        ids32 = ec(nc.sbuf_tensor("ids32", [16, T // 16], i32))
        ids16 = ec(nc.sbuf_tensor("ids16", [128, T // 16], i16))
        gbuf = ec(nc.sbuf_tensor("gbuf", [128, NBUF * A * D], bf16))
        obuf = ec(nc.sbuf_tensor("obuf", [128, NBUF * A * D], f32))
        wrmidx = ec(nc.sbuf_tensor("wrmidx", [128, 1], i16))
        wrmdst = ec(nc.sbuf_tensor("wrmdst", [128, D], bf16))
        s_ids = ec(nc.semaphore("s_ids"))    # ids load
        s_cast = ec(nc.semaphore("s_cast"))  # DVE cast
        s_rep = ec(nc.semaphore("s_rep"))    # ids16 replicate copies
        s_wrm = ec(nc.semaphore("s_wrm"))    # warm-up gather
        s_g = [ec(nc.semaphore(f"s_g{i}")) for i in range(NBUF)]  # gathers
        s_u = [ec(nc.semaphore(f"s_u{i}")) for i in range(NBUF)]  # upconverts
        s_w = [ec(nc.semaphore(f"s_w{i}")) for i in range(NBUF)]  # out writes
        block = ec(nc.Block(no_gpsimd_drain=True))

        def upconvert(e: bass.BassEngine, is_vector: bool):
            # chunk m -> buffer h=m%NBUF; h parity fixed => engine fixed
            for m in range(NCH):
                h, r = m % NBUF, m // NBUF
                if (h % 2 == 0) != is_vector:
                    continue
                e.wait_ge(s_g[h], 16 * (r + 1))
                if r > 0:
                    e.wait_ge(s_w[h], 16 * r)
                src = gbuf[:, h * A * D:(h + 1) * A * D]
                dst = obuf[:, h * A * D:(h + 1) * A * D]
                (e.tensor_copy(dst, src) if is_vector
                 else e.copy(dst, src)).then_inc(s_u[h], 1)

        @block.vector
        def _(v: bass.BassEngine):
            v.wait_ge(s_ids, 16)
            # cast i32->i16, permuting (cc a) -> (a cc) within each chunk so
            # the gather's wrapped idx order maps partition b to A
            # consecutive tokens.
            v.tensor_copy(
                ids16[0:16, :].rearrange("p (c a cc) -> p c a cc", c=NCH, a=A, cc=CC),
                ids32[:, :].rearrange("p (c cc a) -> p c a cc", c=NCH, a=A, cc=CC),
            ).then_inc(s_cast, 1)
            upconvert(v, True)

        @block.scalar
        def _(sc: bass.BassEngine):
            sc.wait_ge(s_cast, 1)
            for k in range(5, 8):
                sc.dma_start(ids16[16 * k:16 * (k + 1), :], ids16[0:16, :]).then_inc(s_rep, 16)
            upconvert(sc, False)

        @block.sync
        def _(s: bass.BassEngine):
            # ids in one rank-4 DMA:
            #   ids32[p, c*(CHUNK//16) + cc*A + a] = ids[c*CHUNK + cc*16*A + p*A + a]
            with nc.allow_non_contiguous_dma(reason="one-time 128KiB idx load"):
                s.dma_start(
                    ids32[:, :].rearrange("p (c cc a) -> p c cc a", c=NCH, cc=CC, a=A),
                    ids[:].rearrange("(c cc p a) -> p c cc a", p=16, c=NCH, cc=CC, a=A),
                ).then_inc(s_ids, 16)

            # replicate int16 idx tiles to partition groups 1-4 (5-7 on ACT)
            s.wait_ge(s_cast, 1)
            for k in range(1, 5):
                s.dma_start(ids16[16 * k:16 * (k + 1), :], ids16[0:16, :]).then_inc(s_rep, 16)

            # chunk output writes: partition b holds rows b*A..b*A+A-1
            for m in range(NCH):
                h, r = m % NBUF, m // NBUF
                s.wait_ge(s_u[h], r + 1)
                s.dma_start(
                    out[m * CHUNK:(m + 1) * CHUNK, :].rearrange("(b x) e -> b (x e)", x=A),
                    obuf[:, h * A * D:(h + 1) * A * D],
                ).then_inc(s_w[h], 16)
            for h in range(NBUF):
                s.wait_ge(s_w[h], 16 * ((NCH + NBUF - 1 - h) // NBUF))

        @block.gpsimd
        def _(g: bass.BassGpSimd):
            g.load_library(mlp)
            # warm-up 16-idx gather: absorbs the one-time SWDGE/DGE start
            # latency (~9us) while the ids prefix runs on other engines
            g.memset(wrmidx[:, :], 0)
            g.dma_gather(
                wrmdst[:, :].rearrange("p (n e) -> p n e", e=D),
                tbl[:, :], wrmidx[:, :], 16, 16, D,
            ).then_inc(s_wrm, 16)
            g.wait_ge(s_rep, 16 * 7)
            for m in range(NCH):
                h, r = m % NBUF, m // NBUF
                if m >= NBUF:
                    # gbuf[h] is free once round r-1's upconvert has read it
                    g.wait_ge(s_u[h], r)
                g.dma_gather(
                    gbuf[:, h * A * D:(h + 1) * A * D].rearrange("p (n e) -> p n e", e=D),
                    tbl[:, :],
                    ids16[:, m * (CHUNK // 16):(m + 1) * (CHUNK // 16)],
                    CHUNK, CHUNK, D,
                    queue_num=m % _nq,
                ).then_inc(s_g[h], 16)
            g.wait_ge(s_wrm, 16)

    nc.compile()
    return nc


_NC_CACHE: list = [None]


def _get_nc() -> bacc.Bacc:
    if _NC_CACHE[0] is None:
        _NC_CACHE[0] = build_nc()
    return _NC_CACHE[0]


def make_in_maps(**inputs) -> list[dict]:
    ids_full = np.ascontiguousarray(np.asarray(inputs["input_ids"], dtype=np.int32))
    # Host prep: fuse tables, compose flat_map (clip = jnp.take default
    # out-of-bounds semantics), quantize to bf16.
    pieces = [np.asarray(inputs[name], dtype=np.float32) for name, _ in TAB_SPECS]
    pieces.append(np.asarray(inputs["unknown_embed"], dtype=np.float32)[None, :])
    table = np.concatenate(pieces, axis=0)            # [1726, 512]
    fm = np.asarray(inputs["flat_map"], dtype=np.int64)
    tbl_fin = np.take(table, fm, axis=0, mode="clip")  # [1725, 512]
    tbl16 = np.ascontiguousarray(tbl_fin.astype(ml_dtypes.bfloat16))
    in_maps = []
    for c in range(NCORES):
        in_maps.append({
            "tbl": tbl16,
            "ids": ids_full[c * BPC:(c + 1) * BPC, :].reshape(-1).copy(),
        })
    return in_maps


def kernel(**inputs) -> np.ndarray:
    nc = _get_nc()
    in_maps = make_in_maps(**inputs)
    res = run_bass_kernel_spmd(nc, in_maps, list(range(NCORES)))
    outs = [res.results[c]["out"] for c in range(NCORES)]
    return np.concatenate(outs, axis=0).reshape(B, S, D)


def kernel_traced(**inputs):
    """Like kernel() but with NTFF profiling; returns (output, BassKernelResults)."""
    nc = _get_nc()
    in_maps = make_in_maps(**inputs)
    res = run_bass_kernel_spmd(nc, in_maps, list(range(NCORES)), trace=True)
    outs = [res.results[c]["out"] for c in range(NCORES)]
    return np.concatenate(outs, axis=0).reshape(B, S, D), res
